# revision 28
# baseline (speedup 1.0000x reference)
"""Block-diagonal (per-frame) multi-head attention on 8 Trainium2 cores.

Problem: x[2,3200,512] -> QKV proj (H=8 heads, D=64) -> attention masked to
25-token frames (128 frames) -> out[2,3200,512].  N = 3200 = 128*25.

Sharding: 8 (batch, frame) groups; core c handles batch c//4, frames
(c%4)*32..+32  => 800 tokens/core, tiled as 8 x 100 tokens (4 frames).

Layout: host sends x pre-transposed (xT [512, 800]) so every matmul
contracts over the partition dim:
  qT/kT [feat, tok] = W.T @ xT   (lhsT = W slice, rhs = xT)
  v     [tok, feat] = xT.T @ Wv  (lhsT = xT slice, rhs = Wv)
Projections run in f32r (1 cyc/row at free>=256 => 4x over fp32, ~fp32
precision); attention matmuls run in bf16.

q/k live in per-head [69, 800] tiles: rows 0-63 the head's features,
rows 64-68 (quadrant-aligned: engine APs must start at partition
0/32/64/96) carry the rank-5 additive frame mask: row 64 =
-128 * ones x ones, rows 65-68 = +128 * frame indicators.  The score
matmul contracts all 69 rows, so the mask is injected for free.  With
C=128 (exact in bf16) the cancellation s + (-C) + C = s holds to ~2e-5
in fp32 PSUM in ANY accumulation order, and cross-frame scores become
s - 128 <= -120, so exp flushes them to 0 in bf16.  S^T for all 4 heads
of a half lands in ONE [100, 400] PSUM tile -> one exp per unit.
softmax skips max-subtraction (|scores| <~ 8).  v carries an all-ones
65th column per head so PV's last column is the softmax denominator;
normalize = one reciprocal + one broadcast multiply per unit.

v-proj tiles are software-pipelined between attention units so the PE's
in-order queue never stalls on the scores->exp->PV dependency chain.
"""

import numpy as np

B, N, DIN = 2, 3200, 512
H, D = 8, 64
TL, JN = 128, 25
NCORES = 8
TOK = 800      # tokens per core
NT = 8         # token tiles per core
TT = 100       # tokens per tile (4 frames)
MR = 5         # mask rows appended to each q/k head tile (rows 64-68)
BIG = 128.0    # exact in bf16; |scores| << 128 so s-128+128 = s to ~2e-5

# matmul dtype per stage: 'f32' | 'f32r' | 'bf16'
CONFIG = {"proj": "f32r", "qk": "bf16", "pv": "bf16"}
OUT_BF16 = True  # DMA the output in bf16, upcast on host

_CACHE = {}
LAST_RESULT = None  # BassKernelResults of the most recent kernel() call


def _build(cfg):
    import concourse.bacc as bacc
    import concourse.tile as tile
    from concourse import mybir

    f32 = mybir.dt.float32
    bf16 = mybir.dt.bfloat16
    f32r = mybir.dt.float32r
    AF = mybir.ActivationFunctionType
    ALU = mybir.AluOpType

    def io_dt(kind):
        return {"f32": f32, "f32r": f32r, "bf16": bf16}[kind]

    proj_dt, qk_dt, pv_dt = cfg["proj"], cfg["qk"], cfg["pv"]

    nc = bacc.Bacc("TRN2", target_bir_lowering=False, debug=False,
                   num_devices=NCORES)

    # W / xT arrive pre-arranged to SBUF layout: k-chunk k of the
    # contraction dim at cols [k*DIN .. ] / [k*TOK .. ].  wqc carries the
    # (scaled) q bias in its last 4 columns.
    xt_d = nc.dram_tensor("xT", [128, 4 * TOK], io_dt(proj_dt),
                          kind="ExternalInput").ap()
    wqc_d = nc.dram_tensor("wqc", [128, 4 * DIN + 4], io_dt(proj_dt),
                           kind="ExternalInput").ap()
    wk_d = nc.dram_tensor("wk", [128, 4 * DIN], io_dt(proj_dt),
                          kind="ExternalInput").ap()
    wv_d = nc.dram_tensor("wv", [128, 4 * DIN], io_dt(proj_dt),
                          kind="ExternalInput").ap()
    bvb_d = nc.dram_tensor("bvb", [TT, DIN], f32, kind="ExternalInput").ap()
    ma_d = nc.dram_tensor("mA", [MR, H * TOK], io_dt(qk_dt),
                          kind="ExternalInput").ap()
    mb_d = nc.dram_tensor("mB", [MR, H * TOK], io_dt(qk_dt),
                          kind="ExternalInput").ap()
    out_dt = bf16 if OUT_BF16 else f32
    out_d = nc.dram_tensor("out", [TOK, DIN], out_dt,
                           kind="ExternalOutput").ap()

    with tile.TileContext(nc) as tc:
        with (
            tc.tile_pool(name="persist", bufs=1) as pp,
            tc.tile_pool(name="scratch", bufs=2) as sp,
        ):
            # ---- DMA in (batched: each dma_start costs ~680ns of Sync
            # issue time; ~13 large DMAs ordered to match PE consumption —
            # the input stream is HBM-bound end-to-end, so arrival order
            # IS the schedule) ----
            wqc_t = pp.tile([128, 4 * DIN + 4], io_dt(proj_dt), name="wqc",
                            tag="wqc")
            xt_t = pp.tile([128, 4 * TOK], io_dt(proj_dt), name="xt",
                           tag="xt")
            xtv = xt_t.rearrange("p (k t) -> p k t", k=4)
            xt_srcv = xt_d.rearrange("p (k t) -> p k t", k=4)
            k01, k23 = slice(0, 2), slice(2, 4)
            # q-ch0 gate: wqc + xt column-half 0, k-pair granularity
            nc.sync.dma_start(out=wqc_t[:, 0:1024], in_=wqc_d[:, 0:1024])
            nc.sync.dma_start(out=xtv[:, k01, 0:400],
                              in_=xt_srcv[:, k01, 0:400])
            nc.sync.dma_start(out=wqc_t[:, 1024:2052],
                              in_=wqc_d[:, 1024:2052])
            nc.sync.dma_start(out=xtv[:, k23, 0:400],
                              in_=xt_srcv[:, k23, 0:400])
            # q-ch1 gate
            for ks in (k01, k23):
                nc.sync.dma_start(out=xtv[:, ks, 400:800],
                                  in_=xt_srcv[:, ks, 400:800])
            # k-proj gate
            wk_t = pp.tile([128, 4 * DIN], io_dt(proj_dt), name="wk",
                           tag="wk")
            nc.sync.dma_start(out=wk_t[:, 0:1024], in_=wk_d[:, 0:1024])
            nc.sync.dma_start(out=wk_t[:, 1024:2048], in_=wk_d[:, 1024:2048])
            # v-proj gate
            wv_t = pp.tile([128, 4 * DIN], io_dt(proj_dt), name="wv",
                           tag="wv")
            nc.sync.dma_start(out=wv_t[:, 0:1024], in_=wv_d[:, 0:1024])
            nc.sync.dma_start(out=wv_t[:, 1024:2048], in_=wv_d[:, 1024:2048])
            bvb = pp.tile([TT, DIN], f32, name="bvb", tag="bvb")
            nc.sync.dma_start(out=bvb, in_=bvb_d)
            bqc = wqc_t[:, 2048:2052].bitcast(f32)
            wq = [wqc_t[:, k * DIN:(k + 1) * DIN] for k in range(4)]
            wk = [wk_t[:, k * DIN:(k + 1) * DIN] for k in range(4)]
            wv = [wv_t[:, k * DIN:(k + 1) * DIN] for k in range(4)]
            xt = [xt_t[:, k * TOK:(k + 1) * TOK] for k in range(4)]

            # q/k in one [69, 8*800] tile each; head h at cols h*800..;
            # rows 0-63 = features, 64-68 = mask factors (host-tiled x8,
            # consumed only once attention starts -> last in the stream)
            QH = pp.tile([64 + MR, H * TOK], io_dt(qk_dt), name="QH",
                         tag="QH")
            KH = pp.tile([64 + MR, H * TOK], io_dt(qk_dt), name="KH",
                         tag="KH")
            nc.sync.dma_start(out=QH[64:64 + MR, :], in_=ma_d)
            nc.sync.dma_start(out=KH[64:64 + MR, :], in_=mb_d)
            qh = [QH[:, h * TOK:(h + 1) * TOK] for h in range(H)]
            kh = [KH[:, h * TOK:(h + 1) * TOK] for h in range(H)]

            # ---- persistent activations ----
            # v with 65 columns per head: col h*65+64 is all-ones so the PV
            # matmul also produces the softmax denominator in its last column
            vt = [pp.tile([TT, H * (D + 1)], io_dt(pv_dt), name=f"vt{t}",
                          tag=f"vt{t}") for t in range(NT)]
            ot = [pp.tile([TT, DIN], out_dt, name=f"ot{t}", tag=f"ot{t}")
                  for t in range(NT)]

            # ---- q^T / k^T projections: psum[feat, tok] ----
            # q copies ride the scalar engine (bias add via activation);
            # k copies go to the proj-phase-idle vector engine (GPSIMD
            # cannot read PSUM).  bk is dropped entirely: q.(k + bk) adds
            # a j-independent term to every score row, which softmax
            # cancels exactly.
            with tc.tile_pool(name="ppsum", bufs=4, space="PSUM") as pps:
                for (w, bc, dsth) in ((wq, bqc, qh), (wk, None, kh)):
                    for ch in range(2):
                        csl = slice(ch * 400, (ch + 1) * 400)
                        for ft in range(4):
                            fsl = slice(ft * 128, (ft + 1) * 128)
                            acc = pps.tile([128, 400], f32, name="pacc",
                                           tag="p", bufs=4)
                            for k in range(4):
                                nc.tensor.matmul(
                                    acc[:], w[k][:, fsl], xt[k][:, csl],
                                    start=(k == 0), stop=(k == 3))
                            for half in range(2):
                                dst = dsth[2 * ft + half][0:64, csl]
                                src = acc[64 * half:64 * half + 64, :]
                                if bc is not None:
                                    nc.scalar.activation(
                                        dst, src, AF.Identity,
                                        bias=bc[64 * half:64 * half + 64,
                                                ft:ft + 1])
                                else:
                                    nc.vector.tensor_copy(dst, src)

            # ---- v projection (pipelined into attention) + attention ----
            with (
                tc.tile_pool(name="vpsum", bufs=2, space="PSUM") as vps,
                tc.tile_pool(name="apsum", bufs=4, space="PSUM") as aps,
            ):
                def emit_v(t):
                    tsl = slice(t * TT, (t + 1) * TT)
                    acc = vps.tile([TT, DIN], f32, name="vacc", tag="v",
                                   bufs=2)
                    for k in range(4):
                        nc.tensor.matmul(acc[:], xt[k][:, tsl], wv[k][:],
                                         start=(k == 0), stop=(k == 3))
                    vdat = vt[t].rearrange("p (h c) -> p h c",
                                           c=D + 1)[:, :, :D]
                    vones = vt[t].rearrange("p (h c) -> p h c",
                                            c=D + 1)[:, :, D:D + 1]
                    nc.vector.scalar_tensor_tensor(
                        vdat, acc.rearrange("p (h c) -> p h c", c=D), 0.0,
                        bvb.rearrange("p (h c) -> p h c", c=D),
                        op0=ALU.add, op1=ALU.add)
                    nc.vector.tensor_scalar_max(vdat, vdat, 0.0)
                    nc.vector.memset(vones, 1.0)

                emit_v(0)
                emit_v(1)
                for u in range(2 * NT):
                    t, hg = u // 2, u % 2
                    tsl = slice(t * TT, (t + 1) * TT)
                    heads = [hg * 4 + i for i in range(4)]
                    st = aps.tile([TT, 4 * TT], f32, name="st", tag="s",
                                  bufs=4)
                    for i, h in enumerate(heads):
                        nc.tensor.matmul(
                            st[:, i * TT:(i + 1) * TT],
                            kh[h][:, tsl], qh[h][:, tsl],
                            start=True, stop=True, skip_group_check=True)
                    # keep the PE queue fed while exp runs on ACT: one
                    # v-proj tile every other unit (v(t) lands >= 3 units
                    # before att unit 2t consumes it)
                    if u % 2 == 1 and u // 2 + 2 < NT:
                        emit_v(u // 2 + 2)
                    et = sp.tile([TT, 4 * TT], io_dt(pv_dt), name="et",
                                 tag="et", bufs=4)
                    nc.scalar.activation(et[:], st[:], AF.Exp)

                    pv4 = aps.tile([TT, 4 * (D + 1)], f32, name="pv4",
                                   tag="pv", bufs=2)
                    for i, h in enumerate(heads):
                        nc.tensor.matmul(
                            pv4[:, i * (D + 1):(i + 1) * (D + 1)],
                            et[:, i * TT:(i + 1) * TT],
                            vt[t][:, h * (D + 1):(h + 1) * (D + 1)],
                            start=True, stop=True, skip_group_check=True)
                    pvv = pv4.rearrange("p (h c) -> p h c", c=D + 1)
                    rc4 = sp.tile([TT, 4], f32, name="rc4", tag="rc", bufs=4)
                    nc.vector.reciprocal(rc4[:], pvv[:, :, D:D + 1])
                    otv = ot[t].rearrange("p (h c) -> p h c",
                                          c=D)[:, hg * 4:hg * 4 + 4, :]
                    nc.vector.tensor_mul(
                        otv, pvv[:, :, :D],
                        rc4.rearrange("p (h o) -> p h o",
                                      o=1).broadcast_to((TT, 4, D)))
                    if hg == 1:
                        nc.sync.dma_start(out=out_d[tsl, :], in_=ot[t][:])

    nc.compile()
    return nc


def _prep_inputs(x, Wq, bq, Wk, bk, Wv, bv, cfg):
    import ml_dtypes

    x = np.asarray(x, np.float32)
    Wq = np.asarray(Wq, np.float32)
    bq = np.asarray(bq, np.float32)
    Wk = np.asarray(Wk, np.float32)
    bk = np.asarray(bk, np.float32)
    Wv = np.asarray(Wv, np.float32)
    bv = np.asarray(bv, np.float32)

    scale = 1.0 / np.sqrt(np.float32(D))  # 1/8, exact
    wq_s = (Wq * scale).astype(np.float32)
    bq_s = (bq * scale).astype(np.float32)

    io_np = ml_dtypes.bfloat16 if cfg["proj"] == "bf16" else np.float32
    qk_np = ml_dtypes.bfloat16 if cfg["qk"] == "bf16" else np.float32
    xT = np.ascontiguousarray(x.transpose(0, 2, 1))  # [B, DIN, N]

    def arrange(w):  # [512, X] -> SBUF layout [128, 4*X] (k-chunk cols)
        X = w.shape[1]
        return w.reshape(4, 128, X).transpose(1, 0, 2).reshape(128, 4 * X)

    # wqc = arranged wq ++ bqc (f32r shares the f32 bit layout)
    assert cfg["proj"] != "bf16", "wqc concat assumes 4-byte proj dtype"
    bqc = bq_s.reshape(4, 128).T  # [128, 4]
    wqc = np.ascontiguousarray(np.concatenate(
        [arrange(wq_s), bqc], axis=1, dtype=np.float32))
    bvb = np.ascontiguousarray(np.tile(bv[None, :], (TT, 1)))

    # rank-5 factors of the additive frame mask, periodic per 100-token
    # tile: row0 = ones / -BIG, rows 1+f = frame-f indicator / +BIG.
    # They sit at rows 64-68 of every q/k head tile and cancel to ~2e-5
    # in fp32 PSUM for same-frame pairs (BIG=128, |s| << 128).
    big = qk_np(BIG)
    frame = (np.arange(TOK) % TT) // JN  # local frame id 0..3
    mA = np.zeros((MR, TOK), qk_np)
    mB = np.zeros((MR, TOK), qk_np)
    mA[0, :] = qk_np(1)
    mB[0, :] = -big
    for f in range(4):
        mA[1 + f, frame == f] = qk_np(1)
        mB[1 + f, frame == f] = big
    mA = np.ascontiguousarray(np.tile(mA, (1, H)))  # replicated per head
    mB = np.ascontiguousarray(np.tile(mB, (1, H)))

    in_maps = []
    for c in range(NCORES):
        b, fb = c // 4, c % 4
        in_maps.append({
            "xT": np.ascontiguousarray(arrange(
                xT[b, :, fb * TOK:(fb + 1) * TOK])).astype(io_np),
            "wqc": wqc,
            "wk": np.ascontiguousarray(arrange(Wk)).astype(io_np),
            "wv": np.ascontiguousarray(arrange(Wv)).astype(io_np),
            "bvb": bvb,
            "mA": mA, "mB": mB,
        })
    return in_maps


def kernel(x, Wq, bq, Wk, bk, Wv, bv, att_heads=H, latent_dim=D,
           time_len=TL, joint_num=JN, **_):
    from concourse.bass_utils import run_bass_kernel_spmd

    cfg = tuple(sorted(CONFIG.items()))
    if cfg not in _CACHE:
        _CACHE[cfg] = _build(CONFIG)
    nc = _CACHE[cfg]

    in_maps = _prep_inputs(x, Wq, bq, Wk, bk, Wv, bv, CONFIG)
    res = run_bass_kernel_spmd(nc, in_maps, core_ids=list(range(NCORES)))
    global LAST_RESULT
    LAST_RESULT = res

    out = np.empty((B, N, DIN), np.float32)
    for c in range(NCORES):
        b, fb = c // 4, c % 4
        out[b, fb * TOK:(fb + 1) * TOK, :] = np.asarray(
            res.results[c]["out"]).astype(np.float32)
    return out
